# revision 2
# baseline (speedup 1.0000x reference)
"""Self-contained BitFeedForward (BitNet b1.58 FFN) Trainium2 kernel.

kernel(x, w1, w2) takes the FULL inputs (x [4,2048,2048] f32, w1 [8192,2048],
w2 [2048,8192]) and returns the full output [4,2048,2048] f32.

Strategy: pure data-parallel over the 8192 tokens across 8 NeuronCores (1024
tokens each); each core runs the whole FFN on its token shard. The only
cross-core communication is a 64-byte AllReduce that combines the per-core
partial sums for mean(|w1|)/mean(|w2|) (each core reduces 1/8 of each weight).

Numerics (matches the BitNet reference):
- activation quant: q = round(x*s), s = 127/clip(rowmax|x|,1e-5); |q| <= 127
  so q is exact in bf16, and int-valued bf16 matmuls accumulated in fp32 PSUM
  are exact.
- weight quant: tern = clip(round(w*s1),-1,1) computed as
  round(clamp(w*s1, +-1.49999988)) (identical incl. round-half-even corners).
- round() = fp32 magic-number trick (+-1.5*2^23, RNE) == jnp.round.
"""

from contextlib import ExitStack

import numpy as np

import concourse.bass as bass
import concourse.bacc as bacc
import concourse.tile as tile
from concourse import mybir
from concourse.masks import make_identity
from concourse.bass_utils import run_bass_kernel_spmd
from concourse.vector_clock import ScopedClock, VectorClock

F32 = mybir.dt.float32
BF16 = mybir.dt.bfloat16
AX = mybir.AxisListType
OP = mybir.AluOpType
AF = mybir.ActivationFunctionType

MAGIC = 1.5 * 2**23  # fp32 round-to-nearest-even magic constant
CLIP = 1.49999988    # largest fp32 < 1.5
EPS = 1e-5
INV127 = 1.0 / 127.0

N_CORES = 8
B_, S_, D_, H_ = 4, 2048, 2048, 8192
T_ = (B_ * S_) // N_CORES  # tokens per core


class PatchedTileContext(tile.TileContext):
    """Workaround for walrus 'Too many sync wait commands' on the kernel-tail
    drain: emit one SP nop per busy proc (single sem wait each) instead of
    piling every wait onto the one drain CTRL instruction."""

    def _drain_and_barrier(self, tick_clock, wait_clock):
        gc = tick_clock.global_clock
        n = len(gc)
        for proc in range(n):
            tick = gc[proc]
            if tick <= 0:
                continue
            vc = VectorClock([0] * n)
            vc.require_at_least(proc, tick)
            nop_inst = self.nc.sync.nop(nofuse=True, hint=f"drain_wait_p{proc}")
            wait_clock.add_sem_waits(nop_inst.ins, ScopedClock({None: vc}))
        self.nc.sync.drain()
        self.nc.all_engine_barrier()
        assert self.sems is not None
        popped = self.nc._tile_sem_poison_stack.pop()
        assert popped is self._sem_poison
        self.nc.clear_and_free_semaphores(list(self.sems.allocated().values()))
        self.nc.all_engine_barrier()


def build_kernel(T, D, H, n_cores, n_weight_elems=None):
    """Build the per-core SPMD kernel (see module docstring)."""
    Tt = T // 128          # token tiles
    Dk = D // 128          # k-tiles of D (phase-1 contraction)
    HC = 512               # phase-1 H chunk (one PSUM bank of f32)
    NC1 = H // HC
    Hk = H // 128          # H k-tiles (phase-2 contraction)
    QC = min(1024, H)      # phase-1.5 read chunk
    NQ = H // QC
    SL = min(512, H)       # phase-2 w2 k-slab
    SLk = SL // 128
    NSL = H // SL
    DC = min(512, D)       # phase-2 D chunk (PSUM bank)
    ND = D // DC
    HSn = H // n_cores
    DSn = D // n_cores
    if n_weight_elems is None:
        n_weight_elems = H * D

    nc = bacc.Bacc("TRN2", target_bir_lowering=False, debug=False,
                   num_devices=n_cores)

    x_ap = nc.dram_tensor("x", [T, D], F32, kind="ExternalInput").ap()
    w1t_ap = nc.dram_tensor("w1t", [D, H], F32, kind="ExternalInput").ap()
    w2t_ap = nc.dram_tensor("w2t", [H, D], F32, kind="ExternalInput").ap()
    w1s_ap = nc.dram_tensor("w1s", [D, HSn], F32, kind="ExternalInput").ap()
    w2s_ap = nc.dram_tensor("w2s", [H, DSn], F32, kind="ExternalInput").ap()
    out_ap = nc.dram_tensor("out", [T, D], F32, kind="ExternalOutput").ap()

    # DRAM views with the partition dim innermost for SBUF-shaped DMA
    w1t_v = w1t_ap.rearrange("(k p) h -> p k h", p=128)
    w2t_v = w2t_ap.rearrange("(s p) d -> p s d", p=128)
    w1s_v = w1s_ap.rearrange("(k p) h -> p k h", p=128)
    w2s_v = w2s_ap.rearrange("(s p) d -> p s d", p=128)

    with tile.TileContext(nc) as tc:
        with ExitStack() as ctx:
            persist = ctx.enter_context(tc.tile_pool(name="persist", bufs=1))
            dram = ctx.enter_context(
                tc.tile_pool(name="dram", bufs=1, space="DRAM"))
            psum = ctx.enter_context(
                tc.tile_pool(name="psum", bufs=8, space="PSUM"))

            def ptile():
                return psum.tile([128, 512], F32, tag="ps", name="ps")

            ident = persist.tile([128, 128], F32, tag="ident")
            make_identity(nc, ident[:])
            magicv = persist.tile([128, 1], F32, tag="magicv")
            nc.gpsimd.memset(magicv[:], MAGIC)
            ones_col = persist.tile([128, 1], F32, tag="ones_col")
            nc.gpsimd.memset(ones_col[:], 1.0)
            ones_row = persist.tile([1, 128], F32, tag="ones_row")
            nc.gpsimd.memset(ones_row[:], 1.0)

            # cols per t: 0=sx 1=invsx 2=deq1 3=runmax 4=sh 5=invsh 6=deq2
            pertok = persist.tile([128, 8 * Tt], F32, tag="pertok")
            partials = persist.tile([128, 8], F32, tag="partials")
            red8 = persist.tile([1, 8], F32, tag="red8")
            tot2 = persist.tile([1, 2], F32, tag="tot2")
            cst = persist.tile([1, 16], F32, tag="cst")
            ccr = persist.tile([1, 16], F32, tag="ccr")
            vals = persist.tile([1, 4], F32, tag="vals")
            bcast = persist.tile([128, 4], F32, tag="bcast")

            hbuf = dram.tile([T, H], F32, tag="hbuf")
            ccin = dram.tile([1, 16], F32, tag="ccin")
            ccout = dram.tile([1, 16], F32, tag="ccout")

            S1 = bcast[:, 0:1]
            S2 = bcast[:, 1:2]
            M1W = bcast[:, 2:3]
            M2W = bcast[:, 3:4]

            with ExitStack() as ctx1:
                pool_x = ctx1.enter_context(
                    tc.tile_pool(name="pool_x", bufs=1))
                xqt = pool_x.tile([128, Tt * Dk * 128], BF16, tag="xqt")

                # ---- phase 0a: x load, scales, quantize, transpose ----
                with ExitStack() as ctx0:
                    pool_rd0 = ctx0.enter_context(
                        tc.tile_pool(name="pool_rd0", bufs=2))
                    pool_st0 = ctx0.enter_context(
                        tc.tile_pool(name="pool_st0", bufs=2))
                    for t in range(Tt):
                        xt = pool_rd0.tile([128, D], F32, tag="xt")
                        nc.sync.dma_start(xt[:],
                                          x_ap[t * 128:(t + 1) * 128, :])
                        sx = pertok[:, 8 * t + 0:8 * t + 1]
                        invsx = pertok[:, 8 * t + 1:8 * t + 2]
                        mx = pool_st0.tile([128, 1], F32, tag="mx")
                        nc.vector.tensor_reduce(mx[:], xt[:], axis=AX.X,
                                                op=OP.max,
                                                apply_absolute_value=True)
                        nc.vector.tensor_scalar(invsx, mx[:], EPS, INV127,
                                                OP.max, OP.mult)
                        nc.vector.reciprocal(sx, invsx)
                        qx = pool_st0.tile([128, D], F32, tag="qx")
                        nc.scalar.activation(qx[:], xt[:], AF.Identity,
                                             bias=magicv[:, 0:1], scale=sx)
                        for kk in range(0, Dk, 4):
                            ps = ptile()
                            for k4 in range(min(4, Dk - kk)):
                                k = kk + k4
                                nc.tensor.transpose(
                                    ps[:, k4 * 128:(k4 + 1) * 128],
                                    qx[:, k * 128:(k + 1) * 128], ident[:])
                                nc.scalar.activation(
                                    xqt[:, (t * Dk + k) * 128:
                                        (t * Dk + k) * 128 + 128],
                                    ps[:, k4 * 128:(k4 + 1) * 128],
                                    AF.Copy, bias=-MAGIC)

                    # ---- phase 0b: weight scale partials + AllReduce ----
                    for i, (src_v, nk, ncols) in enumerate(
                            ((w1s_v, Dk, HSn), (w2s_v, Hk, DSn))):
                        npiece = 4
                        if nk >= npiece:
                            assert nk % npiece == 0
                            kstep = nk // npiece
                            for p in range(npiece):
                                wt = pool_rd0.tile([128, kstep, ncols], F32,
                                                   tag="p0")
                                nc.sync.dma_start(
                                    wt[:],
                                    src_v[:, p * kstep:(p + 1) * kstep, :])
                                nc.vector.tensor_reduce(
                                    partials[:, 4 * i + p:4 * i + p + 1],
                                    wt[:].rearrange("p a b -> p (a b)"),
                                    axis=AX.X, op=OP.add,
                                    apply_absolute_value=True)
                        else:
                            assert ncols % npiece == 0
                            cstep = ncols // npiece
                            for p in range(npiece):
                                wt = pool_rd0.tile([128, nk, cstep], F32,
                                                   tag="p0")
                                nc.sync.dma_start(
                                    wt[:],
                                    src_v[:, :, p * cstep:(p + 1) * cstep])
                                nc.vector.tensor_reduce(
                                    partials[:, 4 * i + p:4 * i + p + 1],
                                    wt[:].rearrange("p a b -> p (a b)"),
                                    axis=AX.X, op=OP.add,
                                    apply_absolute_value=True)
                    pss = ptile()
                    nc.tensor.matmul(pss[0:1, 0:8], ones_col[:], partials[:],
                                     start=True, stop=True)
                    nc.scalar.copy(red8[:], pss[0:1, 0:8])
                    nc.vector.tensor_reduce(tot2[:, 0:1], red8[:, 0:4],
                                            axis=AX.X, op=OP.add)
                    nc.vector.tensor_reduce(tot2[:, 1:2], red8[:, 4:8],
                                            axis=AX.X, op=OP.add)
                    nc.gpsimd.memset(cst[:], 0.0)
                    nc.vector.tensor_copy(cst[:, 0:2], tot2[:])
                    nc.sync.dma_start(ccin[:], cst[:])
                    nc.gpsimd.collective_compute(
                        "AllReduce", OP.add,
                        replica_groups=[list(range(n_cores))],
                        ins=[ccin.opt()], outs=[ccout.opt()])
                    nc.sync.dma_start(ccr[:], ccout[:])
                    nc.vector.tensor_scalar(vals[:, 2:4], ccr[:, 0:2],
                                            1.0 / float(n_weight_elems), EPS,
                                            OP.mult, OP.max)
                    nc.vector.reciprocal(vals[:, 0:2], vals[:, 2:4])
                    psb = ptile()
                    nc.tensor.matmul(psb[:, 0:4], ones_row[:], vals[:],
                                     start=True, stop=True)
                    nc.scalar.copy(bcast[:], psb[:, 0:4])

                for t in range(Tt):
                    nc.vector.tensor_scalar(pertok[:, 8 * t + 2:8 * t + 3],
                                            pertok[:, 8 * t + 1:8 * t + 2],
                                            M1W, None, OP.mult)

                # ---- phase 1: h = gelu(deq1 * (xq @ w1q^T)), rowmax ----
                with ExitStack() as ctxp1:
                    pool_w1 = ctxp1.enter_context(
                        tc.tile_pool(name="pool_w1", bufs=2))
                    pool_wq1 = ctxp1.enter_context(
                        tc.tile_pool(name="pool_wq1", bufs=2))
                    pool_st1 = ctxp1.enter_context(
                        tc.tile_pool(name="pool_st1", bufs=4))
                    for hc in range(NC1):
                        w1f = pool_w1.tile([128, Dk, HC], F32, tag="w1f")
                        nc.sync.dma_start(
                            w1f[:], w1t_v[:, :, hc * HC:(hc + 1) * HC])
                        nc.gpsimd.tensor_scalar(w1f[:], w1f[:], S1, CLIP,
                                                OP.mult, OP.min)
                        nc.vector.tensor_scalar(w1f[:], w1f[:], -CLIP, MAGIC,
                                                OP.max, OP.add)
                        w1q = pool_wq1.tile([128, Dk, HC], BF16, tag="w1q")
                        nc.vector.tensor_scalar(w1q[:], w1f[:], MAGIC, None,
                                                OP.subtract)
                        for t in range(Tt):
                            ps = ptile()
                            for k in range(Dk):
                                nc.tensor.matmul(
                                    ps[:, 0:HC],
                                    xqt[:, (t * Dk + k) * 128:
                                        (t * Dk + k) * 128 + 128],
                                    w1q[:, k, :],
                                    start=(k == 0), stop=(k == Dk - 1))
                            hsb = pool_st1.tile([128, HC], F32, tag="hsb")
                            nc.scalar.activation(
                                hsb[:], ps[:, 0:HC], AF.Gelu,
                                scale=pertok[:, 8 * t + 2:8 * t + 3])
                            mx1 = pool_st1.tile([128, 1], F32, tag="mx1")
                            nc.vector.tensor_reduce(
                                mx1[:], hsb[:], axis=AX.X, op=OP.max,
                                apply_absolute_value=True)
                            runmax = pertok[:, 8 * t + 3:8 * t + 4]
                            if hc == 0:
                                nc.vector.tensor_copy(runmax, mx1[:])
                            else:
                                nc.vector.tensor_max(runmax, runmax, mx1[:])
                            nc.sync.dma_start(
                                hbuf[t * 128:(t + 1) * 128,
                                     hc * HC:(hc + 1) * HC], hsb[:])

            # ---- phase 1.5 + 2 (xqt pool closed; big hqt pool opens) ----
            with ExitStack() as ctx2:
                pool_h = ctx2.enter_context(
                    tc.tile_pool(name="pool_h", bufs=1))
                hqt = pool_h.tile([128, Hk * Tt * 128], BF16, tag="hqt")

                with ExitStack() as ctx15:
                    pool_rd15 = ctx15.enter_context(
                        tc.tile_pool(name="pool_rd15", bufs=2))
                    for t in range(Tt):
                        sh = pertok[:, 8 * t + 4:8 * t + 5]
                        invsh = pertok[:, 8 * t + 5:8 * t + 6]
                        deq2 = pertok[:, 8 * t + 6:8 * t + 7]
                        runmax = pertok[:, 8 * t + 3:8 * t + 4]
                        nc.vector.tensor_scalar(invsh, runmax, EPS, INV127,
                                                OP.max, OP.mult)
                        nc.vector.reciprocal(sh, invsh)
                        nc.vector.tensor_scalar(deq2, invsh, M2W, None,
                                                OP.mult)
                        for qc in range(NQ):
                            hrd = pool_rd15.tile([128, QC], F32, tag="hrd")
                            nc.sync.dma_start(
                                hrd[:], hbuf[t * 128:(t + 1) * 128,
                                             qc * QC:(qc + 1) * QC])
                            qtl = pool_rd15.tile([128, QC], F32, tag="qtl")
                            nc.scalar.activation(qtl[:], hrd[:], AF.Identity,
                                                 bias=magicv[:, 0:1],
                                                 scale=sh)
                            for jj in range(0, QC // 128, 4):
                                ps = ptile()
                                for j4 in range(min(4, QC // 128 - jj)):
                                    j = jj + j4
                                    k2 = qc * (QC // 128) + j
                                    nc.tensor.transpose(
                                        ps[:, j4 * 128:(j4 + 1) * 128],
                                        qtl[:, j * 128:(j + 1) * 128],
                                        ident[:])
                                    nc.scalar.activation(
                                        hqt[:, (k2 * Tt + t) * 128:
                                            (k2 * Tt + t) * 128 + 128],
                                        ps[:, j4 * 128:(j4 + 1) * 128],
                                        AF.Copy, bias=-MAGIC)

                # ---- phase 2: out = deq2 * (hq @ w2q^T) ----
                with ExitStack() as ctxp2:
                    pool_w2 = ctxp2.enter_context(
                        tc.tile_pool(name="pool_w2", bufs=2))
                    pool_wq2 = ctxp2.enter_context(
                        tc.tile_pool(name="pool_wq2", bufs=2))
                    pool_st2 = ctxp2.enter_context(
                        tc.tile_pool(name="pool_st2", bufs=4))
                    for dc in range(ND):
                        pso = [ptile() for _ in range(Tt)]
                        for sl in range(NSL):
                            w2f = pool_w2.tile([128, SLk, DC], F32, tag="w2f")
                            nc.sync.dma_start(
                                w2f[:], w2t_v[:, sl * SLk:(sl + 1) * SLk,
                                              dc * DC:(dc + 1) * DC])
                            nc.gpsimd.tensor_scalar(w2f[:], w2f[:], S2, CLIP,
                                                    OP.mult, OP.min)
                            nc.vector.tensor_scalar(w2f[:], w2f[:], -CLIP,
                                                    MAGIC, OP.max, OP.add)
                            w2q = pool_wq2.tile([128, SLk, DC], BF16,
                                                tag="w2q")
                            nc.vector.tensor_scalar(w2q[:], w2f[:], MAGIC,
                                                    None, OP.subtract)
                            for t in range(Tt):
                                for kk in range(SLk):
                                    k2 = sl * SLk + kk
                                    nc.tensor.matmul(
                                        pso[t][:, 0:DC],
                                        hqt[:, (k2 * Tt + t) * 128:
                                            (k2 * Tt + t) * 128 + 128],
                                        w2q[:, kk, :],
                                        start=(k2 == 0), stop=(k2 == Hk - 1),
                                        skip_group_check=True)
                        for t in range(Tt):
                            osb = pool_st2.tile([128, DC], F32, tag="osb")
                            nc.scalar.activation(
                                osb[:], pso[t][:, 0:DC], AF.Copy,
                                scale=pertok[:, 8 * t + 6:8 * t + 7])
                            nc.sync.dma_start(
                                out_ap[t * 128:(t + 1) * 128,
                                       dc * DC:(dc + 1) * DC], osb[:])

    nc.compile()
    return nc


def shard_inputs(x, w1, w2, n_cores):
    B, S, Dx = x.shape
    T_total = B * S
    T = T_total // n_cores
    xf = np.ascontiguousarray(x.reshape(T_total, Dx))
    w1t = np.ascontiguousarray(w1.T)
    w2t = np.ascontiguousarray(w2.T)
    H = w1.shape[0]
    D = Dx
    HSn = H // n_cores
    DSn = D // n_cores
    in_maps = []
    for i in range(n_cores):
        in_maps.append({
            "x": np.ascontiguousarray(xf[i * T:(i + 1) * T]),
            "w1t": w1t,
            "w2t": w2t,
            "w1s": np.ascontiguousarray(w1t[:, i * HSn:(i + 1) * HSn]),
            "w2s": np.ascontiguousarray(w2t[:, i * DSn:(i + 1) * DSn]),
        })
    return in_maps, T


_NC_CACHE = {}


def _get_nc():
    key = (T_, D_, H_, N_CORES)
    if key not in _NC_CACHE:
        _NC_CACHE[key] = build_kernel(T_, D_, H_, N_CORES)
    return _NC_CACHE[key]


def run_spmd(x, w1, w2, **run_kwargs):
    """Shard, run on the 8 cores, gather. Returns (out, BassKernelResults)."""
    x = np.asarray(x, dtype=np.float32)
    w1 = np.asarray(w1, dtype=np.float32)
    w2 = np.asarray(w2, dtype=np.float32)
    B, S, D = x.shape
    nc = _get_nc()
    in_maps, T = shard_inputs(x, w1, w2, N_CORES)
    res = run_bass_kernel_spmd(nc, in_maps, list(range(N_CORES)), **run_kwargs)
    outs = [res.results[i]["out"] for i in range(N_CORES)]
    out = np.concatenate(outs, axis=0).reshape(B, S, D).astype(np.float32)
    return out, res


def kernel(x, w1, w2):
    out, _ = run_spmd(x, w1, w2)
    return out
